# revision 24
# baseline (speedup 1.0000x reference)
"""Trainium2 Bass kernel for nn_CliffordFourierHead (CGENN-style Clifford net).

Network (per reference): B=1024, IN=256, HID=512, OUT=128, Cl(3,0), 8 blades.
  fcgp1 -> MVSiLU -> channel-wise steerable GP -> MVSiLU -> fcgp2

Strategy (v3):
  - Pure batch data-parallelism over 8 NeuronCores (128 batch rows each).
  - All weights host-transposed to the exact [128, cols] SBUF layout ->
    contiguous multi-KB DMA lines at full HBM bandwidth; negated GP-weight
    copies (for Cayley minus signs) shipped from host too.
  - Weight DMAs for phases 1-4 issued up front; phase 5-6 weights stream
    during phases 3-4 into the space freed by the phase-1-2 pool.
  - One [128, 8*BC] f32 PSUM tile per accumulation group (exactly 2 banks:
    [g0|g1] and [g2|g3] each 512 f32) -> single-op evacuation, 4 groups in
    flight for deep matmul pipelining.
  - Geometric products: mega product tile Q[i,k] = x_i * xr_k on DVE; Cayley
    contraction via TensorE matmuls with strided plane-selection rhs APs.
  - Channel-wise steerable GP as diagonal-matrix matmuls.
  - Normalization/MVSiLU read PSUM directly for squares; affine norm ops on
    ScalarE to offload DVE.

Self-contained: shapes and the Cl(3,0) Cayley table are derived inline.
"""

import contextlib
import math

import numpy as np

NCORES = 8
B, NIN, HID, NOUT = 1024, 256, 512, 128
BC = B // NCORES  # 128 batch rows per core
NB = 8
KT_IN, KT_HID = NIN // 128, HID // 128  # 2, 4
MT_IN, MT_HID, MT_OUT = NIN // 128, HID // 128, NOUT // 128  # 2, 4, 1
GRADE_SLICES = [(0, 1), (1, 4), (4, 7), (7, 8)]
EPS = 1e-6
ISQ2 = 1.0 / math.sqrt(2.0)


def _build_cayley():
    masks = sorted(range(NB), key=lambda m: (bin(m).count("1"), m))
    pos = {m: i for i, m in enumerate(masks)}
    cay = np.zeros((NB, NB, NB), dtype=np.float32)
    for i, mi in enumerate(masks):
        for k, mk in enumerate(masks):
            a, s = mi >> 1, 0
            while a:
                s += bin(a & mk).count("1")
                a >>= 1
            cay[i, pos[mi ^ mk], k] = -1.0 if (s & 1) else 1.0
    triples = []
    for gi in range(4):
        for gj in range(4):
            for gk in range(4):
                (i0, i1), (j0, j1), (k0, k1) = (
                    GRADE_SLICES[gi], GRADE_SLICES[gj], GRADE_SLICES[gk])
                if np.any(cay[i0:i1, j0:j1, k0:k1] != 0):
                    triples.append((gi, gj, gk))
    return cay, triples


CAY, TRIPLES = _build_cayley()
NPATHS = len(TRIPLES)  # 20

# Per triple t: {j: [(i, k, sign), ...]}
TRIPLE_TERMS = []
for t, (gi, gj, gk) in enumerate(TRIPLES):
    (i0, i1), (k0, k1) = GRADE_SLICES[gi], GRADE_SLICES[gk]
    d = {}
    for i in range(i0, i1):
        for k in range(k0, k1):
            j = int(np.nonzero(CAY[i, :, k])[0][0])
            if GRADE_SLICES[gj][0] <= j < GRADE_SLICES[gj][1]:
                d.setdefault(j, []).append((i, k, float(CAY[i, j, k])))
    TRIPLE_TERMS.append(d)


def _build_term_sets():
    """Per triple: list of matmul term-sets (j0, L, plane0, plane_step, sign)."""
    all_sets = []
    for t in range(NPATHS):
        terms = []
        for j, lst in TRIPLE_TERMS[t].items():
            for (i, k, s) in lst:
                terms.append((j, i * 8 + k, s))
        sets = []
        for sgn in (1.0, -1.0):
            pool = sorted(x for x in terms if x[2] == sgn)
            while pool:
                j0, o0, _ = pool.pop(0)
                run = [(j0, o0)]
                step = None
                while True:
                    pick = None
                    for c in pool:
                        if c[0] != run[-1][0] + 1:
                            continue
                        st = c[1] - run[-1][1]
                        if step is None or st == step:
                            pick, pstep = c, st
                            break
                    if pick is None:
                        break
                    step = pstep
                    pool.remove(pick)
                    run.append((pick[0], pick[1]))
                sets.append((run[0][0], len(run), run[0][1], step or 0, sgn))
        all_sets.append(sets)
    return all_sets


TERM_SETS = _build_term_sets()
# Pre-reduced planes: t4 (1,0,1) planes 9,18,27 -> plane 64 (+);
# t10 (2,0,2) planes 36,45,54 -> plane 65 (negated weight handles sign).
TERM_SETS[4] = [(0, 1, 64, 0, 1.0)]
TERM_SETS[10] = [(0, 1, 65, 0, -1.0)]
NQPL = 66
NEG_TRIPLES = sorted({t for t in range(NPATHS)
                      if any(s[4] < 0 for s in TERM_SETS[t])})
NEG_SLOT = {t: n for n, t in enumerate(NEG_TRIPLES)}
NNEG = len(NEG_TRIPLES)


# ----------------------------------------------------------------------------
# Host-side prep: everything in final [128, cols] SBUF layout
# ----------------------------------------------------------------------------
def prep_in_maps(inputs):
    f16, f32 = np.float16, np.float32

    def lin_w(w, scale=1.0):
        # [m, n, 4] -> [128, nkt*4*m]
        m, n, _ = np.asarray(w).shape
        wt = np.transpose(np.asarray(w, f32), (1, 2, 0))  # [n, 4, m]
        wt = wt.reshape(n // 128, 128, 4, m).transpose(1, 0, 2, 3)
        return np.ascontiguousarray((wt * scale).reshape(128, -1)).astype(f16)

    def gp_w(w, scale):
        # [m, n, 20] -> pos [128, nkt*20*m], neg [128, nkt*12*m]
        m, n, _ = np.asarray(w).shape
        wt = np.transpose(np.asarray(w, f32), (1, 2, 0)) * scale  # [n, 20, m]
        wt = wt.reshape(n // 128, 128, NPATHS, m).transpose(1, 0, 2, 3)
        pos = np.ascontiguousarray(wt.reshape(128, -1)).astype(f16)
        neg = np.ascontiguousarray(
            (-wt[:, :, NEG_TRIPLES, :]).reshape(128, -1)).astype(f16)
        return pos, neg

    def sig(a):
        return 1.0 / (1.0 + np.exp(-np.asarray(a, f32)))

    x = np.asarray(inputs["x"], f32)

    c = {}
    c["lr1w"] = lin_w(inputs["lr1_w"])
    c["ll1w"] = lin_w(inputs["ll1_w"], ISQ2)
    c["lrgw"] = lin_w(inputs["lrg_w"])
    c["llgw"] = lin_w(inputs["llg_w"], ISQ2)
    c["lr2w"] = lin_w(inputs["lr2_w"])
    c["ll2w"] = lin_w(inputs["ll2_w"], ISQ2)
    c["w1w"], c["w1n"] = gp_w(inputs["w1"], ISQ2)
    c["w2w"], c["w2n"] = gp_w(inputs["w2"], ISQ2)

    # channel-wise GP weights as diagonal matrices, [128, ct*20*128] (+neg)
    wg = np.asarray(inputs["wg"], f32) * ISQ2  # [HID, 20]
    wgv = wg.reshape(MT_HID, 128, NPATHS)
    dwg = np.zeros((128, MT_HID, NPATHS, 128), f32)
    idx = np.arange(128)
    for ct in range(MT_HID):
        for t in range(NPATHS):
            dwg[idx, ct, t, idx] = wgv[ct, :, t]
    c["dwg"] = np.ascontiguousarray(dwg.reshape(128, -1)).astype(f16)
    dwgn = -dwg[:, :, NEG_TRIPLES, :]
    c["dwgn"] = np.ascontiguousarray(dwgn.reshape(128, -1)).astype(f16)

    cols = []   # [128, w] blocks, order must match PARAM_LAYOUT

    def addp(arr):
        cols.append(np.asarray(arr, f32).reshape(128, -1))

    for nm, a, kt in (("n1", inputs["n1_a"], KT_IN),
                      ("ng", inputs["ng_a"], KT_HID),
                      ("n2", inputs["n2_a"], KT_HID)):
        sa = sig(a).reshape(kt, 128, 4)
        cb = (1.0 + EPS) - sa
        for u in range(kt):
            addp(sa[u])
            addp(cb[u])
    aa = np.asarray(inputs["act_a"], f32).reshape(MT_HID, 128, 4)
    ab = np.asarray(inputs["act_b"], f32).reshape(MT_HID, 128, 4)
    for u in range(MT_HID):
        addp(aa[u])
        addp(ab[u])
    addp((np.asarray(inputs["ll1_b"], f32) * ISQ2).reshape(MT_HID, 128).T)
    addp((np.asarray(inputs["llg_b"], f32) * ISQ2).reshape(MT_HID, 128).T)
    addp((np.asarray(inputs["ll2_b"], f32) * ISQ2).reshape(MT_OUT, 128).T)
    c["prm"] = np.ascontiguousarray(np.concatenate(cols, axis=1))

    in_maps = []
    for cid in range(NCORES):
        xc = x[cid * BC:(cid + 1) * BC]  # [BC, 256, 8]
        xt = np.transpose(xc, (1, 2, 0)).reshape(KT_IN, 128, NB, BC)
        xt = xt.transpose(1, 0, 2, 3).reshape(128, -1)  # [128, kt*8*BC]
        m = dict(c)
        m["xT"] = np.ascontiguousarray(xt).astype(f16)
        in_maps.append(m)
    return in_maps


def assemble(results):
    out = np.empty((B, NOUT, NB), np.float32)
    for cid in range(NCORES):
        od = np.asarray(results[cid]["outd"])  # [128, 8, BC]
        out[cid * BC:(cid + 1) * BC] = od.transpose(2, 0, 1)
    return out


# ----------------------------------------------------------------------------
# Device program (identical on all 8 cores)
# ----------------------------------------------------------------------------
def build_program():
    import concourse.mybir as mybir
    import concourse.tile as tile
    from concourse import bacc

    dt = mybir.dt
    AF = mybir.ActivationFunctionType
    OP = mybir.AluOpType

    nc = bacc.Bacc("TRN2", target_bir_lowering=False, debug=False,
                   num_devices=NCORES)

    def din(name, cols, dtype=dt.float16):
        return nc.dram_tensor(name, [128, cols], dtype,
                              kind="ExternalInput").ap()

    xT = din("xT", KT_IN * NB * BC)
    lr1w = din("lr1w", KT_IN * 4 * NIN)
    ll1w = din("ll1w", KT_IN * 4 * HID)
    w1w = din("w1w", KT_IN * NPATHS * HID)
    w1n = din("w1n", KT_IN * NNEG * HID)
    lrgw = din("lrgw", KT_HID * 4 * HID)
    llgw = din("llgw", KT_HID * 4 * HID)
    dwg = din("dwg", MT_HID * NPATHS * 128)
    dwgn = din("dwgn", MT_HID * NNEG * 128)
    lr2w = din("lr2w", KT_HID * 4 * HID)
    w2w = din("w2w", KT_HID * NPATHS * NOUT)
    w2n = din("w2n", KT_HID * NNEG * NOUT)
    ll2w = din("ll2w", KT_HID * 4 * NOUT)
    prm = din("prm", 121, dt.float32)
    outd = nc.dram_tensor("outd", [128, NB, BC], dt.float32,
                          kind="ExternalOutput").ap()

    P = lambda j: slice(j * BC, (j + 1) * BC)
    GSL = [slice(j0 * BC, j1 * BC) for (j0, j1) in GRADE_SLICES]

    with tile.TileContext(nc) as tc:
        top = contextlib.ExitStack()
        with top:
            ppool = top.enter_context(tc.tile_pool(name="params", bufs=1))
            w3apool = top.enter_context(tc.tile_pool(name="w_s3a", bufs=1))
            npool = top.enter_context(tc.tile_pool(name="nsc", bufs=2))
            qpool = top.enter_context(tc.tile_pool(name="q", bufs=2))
            pspool = top.enter_context(
                tc.tile_pool(name="psum", bufs=8, space="PSUM"))
            hpool = top.enter_context(tc.tile_pool(name="hacts", bufs=1))
            auxpool = top.enter_context(tc.tile_pool(name="aux", bufs=1))

            def wload(pool, name, src, cols, chunks=1):
                t = pool.tile([128, cols], dt.float16, tag=name, name=name)
                step = cols // chunks
                for ci in range(chunks):
                    nc.sync.dma_start(t[:, ci * step:(ci + 1) * step],
                                      src[:, ci * step:(ci + 1) * step])
                return t

            def lin_slicer(t, mtot):
                def sl(kt, g, mt):
                    base = (kt * 4 + g) * mtot + mt * 128
                    return t[:, base:base + 128]
                return sl

            def gp_slicers(tp, tn, mtot):
                def sl(kt, tt, mt):
                    base = (kt * NPATHS + tt) * mtot + mt * 128
                    return tp[:, base:base + 128]

                def sln(kt, tt, mt):
                    base = (kt * NNEG + NEG_SLOT[tt]) * mtot + mt * 128
                    return tn[:, base:base + 128]
                return sl, sln

            prmt = ppool.tile([128, 121], dt.float32, tag="prm", name="prm")
            nc.sync.dma_start(prmt[:], prm)
            PN1, PNG, PN2, PACT, PB1, PBG, PB2 = 0, 16, 48, 80, 112, 116, 120

            n1sat = {u: prmt[:, PN1 + 8 * u:PN1 + 8 * u + 4]
                     for u in range(KT_IN)}
            n1cbt = {u: prmt[:, PN1 + 8 * u + 4:PN1 + 8 * u + 8]
                     for u in range(KT_IN)}
            ngsat = {u: prmt[:, PNG + 8 * u:PNG + 8 * u + 4]
                     for u in range(KT_HID)}
            ngcbt = {u: prmt[:, PNG + 8 * u + 4:PNG + 8 * u + 8]
                     for u in range(KT_HID)}
            n2sat = {u: prmt[:, PN2 + 8 * u:PN2 + 8 * u + 4]
                     for u in range(KT_HID)}
            n2cbt = {u: prmt[:, PN2 + 8 * u + 4:PN2 + 8 * u + 8]
                     for u in range(KT_HID)}
            actat = {u: prmt[:, PACT + 8 * u:PACT + 8 * u + 4]
                     for u in range(MT_HID)}
            actbt = {u: prmt[:, PACT + 8 * u + 4:PACT + 8 * u + 8]
                     for u in range(MT_HID)}
            b1t = {u: prmt[:, PB1 + u:PB1 + u + 1] for u in range(MT_HID)}
            bgt = {u: prmt[:, PBG + u:PBG + u + 1] for u in range(MT_HID)}
            b2t = {0: prmt[:, PB2:PB2 + 1]}

            GP_SETS_BY_GRADE = {g: [(t, s) for t in range(NPATHS)
                                    if TRIPLES[t][1] == g
                                    for s in TERM_SETS[t]]
                                for g in range(4)}

            class RegionEmitter:
                """start on first / stop on last matmul per psum region."""

                def __init__(self, totals):
                    self.totals = dict(totals)
                    self.seen = {}

                def mm(self, reg, dst, lhs, rhs):
                    i = self.seen.get(reg, 0)
                    nc.tensor.matmul(dst, lhs, rhs, start=(i == 0),
                                     stop=(i == self.totals[reg] - 1))
                    self.seen[reg] = i + 1

                def done(self):
                    assert self.seen == self.totals, (self.seen, self.totals)

            GW = [1, 3, 3, 1]  # blade planes per grade

            def alloc_ps(nm):
                """Four per-grade psum tiles (one bank each) — separate banks
                so interleaved start=True bank-clears stay independent."""
                return [pspool.tile([128, GW[g] * BC], dt.float32, tag="psg",
                                    name=f"ps_{nm}_{g}") for g in range(4)]

            def plane_sel(qpl, o0, L, st):
                if L == 1:
                    return qpl[:, o0:o0 + 1, :]
                last = o0 + st * (L - 1)
                stop = last + 1 if st > 0 else (last - 1 if last >= 1 else None)
                return qpl[:, o0:stop:st, :]

            def build_q(xt, xrt):
                """Mega product tile Q[i*8+k] = x_i * xr_k, [128, 8192]."""
                q = qpool.tile([128, NQPL * BC], dt.float16, tag="Q",
                               name="Q")
                for half in range(2):
                    i0 = half * 4
                    a = xt[:, i0 * BC:(i0 + 4) * BC].rearrange(
                        "p (i u b) -> p i u b", i=4, u=1).broadcast_to(
                        [128, 4, 8, BC])
                    bb = xrt[:].rearrange(
                        "p (u k b) -> p u k b", u=1, k=8).broadcast_to(
                        [128, 4, 8, BC])
                    dst = q[:, i0 * 8 * BC:(i0 + 4) * 8 * BC].rearrange(
                        "p (i k b) -> p i k b", i=4, k=8)
                    nc.vector.tensor_mul(dst, a, bb)
                # pre-reduce t4 (9+18+27 -> 64) and t10 (36+45+54 -> 65)
                qv = q[:].rearrange("p (pl b) -> p pl b", pl=NQPL)
                nc.vector.tensor_add(qv[:, 64:66, :], qv[:, 9:37:27, :],
                                     qv[:, 18:46:27, :])
                nc.vector.tensor_add(qv[:, 64:66, :], qv[:, 64:66, :],
                                     qv[:, 27:55:27, :])
                return q

            def emit_lin(em, psg, wsl, xts, nkt, mt, mtot):
                for kt in range(nkt):
                    for g in range(4):
                        em.mm(g, psg[g][:], wsl(kt, g, mt),
                              xts[kt][:, GSL[g]])

            def emit_gp_kt(em, psg, wsl, wsln, qpl, kt, mt, mtot):
                for g in range(4):
                    gbase = GRADE_SLICES[g][0]
                    for (t, (j0, L, o0, st, sgn)) in GP_SETS_BY_GRADE[g]:
                        lhs = (wsl if sgn > 0 else wsln)(kt, t, mt)
                        r0 = j0 - gbase
                        em.mm(g, psg[g][:, r0 * BC:(r0 + L) * BC],
                              lhs, plane_sel(qpl, o0, L, st))

            def normalize(psg, out, sat, cbt):
                """out = ps / (sa*sqrt(q)+cb). Evacuate psum to SBUF f16
                immediately (frees the banks fast for the next group), then
                run the whole chain from SBUF."""
                raw = npool.tile([128, NB * BC], dt.float16, tag="nraw",
                                 name="nraw")
                sqw = npool.tile([128, 6 * BC], dt.float16, tag="sqw",
                                 name="sqw")
                qw = npool.tile([128, 4 * BC], dt.float16, tag="qw",
                                name="qw")
                nc.scalar.activation(qw[:, 0:BC], psg[0][:], AF.Square)
                nc.scalar.activation(sqw[:, 0:3 * BC], psg[1][:], AF.Square)
                nc.scalar.activation(sqw[:, 3 * BC:], psg[2][:], AF.Square)
                nc.scalar.activation(qw[:, 3 * BC:], psg[3][:], AF.Square)
                nc.scalar.copy(raw[:, 0:BC], psg[0][:])
                nc.scalar.copy(raw[:, BC:4 * BC], psg[1][:])
                nc.scalar.copy(raw[:, 4 * BC:7 * BC], psg[2][:])
                nc.scalar.copy(raw[:, 7 * BC:], psg[3][:])
                sqp = sqw[:].rearrange("p (pl b) -> p pl b", pl=6)
                qp = qw[:, BC:3 * BC].rearrange("p (pl b) -> p pl b", pl=2)
                nc.vector.tensor_add(qp, sqp[:, 0:4:3, :], sqp[:, 1:5:3, :])
                nc.vector.tensor_add(qp, qp, sqp[:, 2:6:3, :])
                nrmw = npool.tile([128, 4 * BC], dt.float16, tag="nrmw",
                                  name="nrmw")
                nc.scalar.activation(nrmw[:], qw[:], AF.Sqrt)
                dw = npool.tile([128, 4 * BC], dt.float32, tag="dw",
                                name="dw")
                for g in range(4):
                    eng = nc.scalar if g % 2 == 0 else None
                    if eng is not None:
                        eng.activation(dw[:, g * BC:(g + 1) * BC],
                                       nrmw[:, g * BC:(g + 1) * BC],
                                       AF.Identity, bias=cbt[:, g:g + 1],
                                       scale=sat[:, g:g + 1])
                    else:
                        nc.vector.tensor_scalar(
                            dw[:, g * BC:(g + 1) * BC],
                            nrmw[:, g * BC:(g + 1) * BC],
                            sat[:, g:g + 1], cbt[:, g:g + 1],
                            OP.mult, OP.add)
                rw = npool.tile([128, 4 * BC], dt.float32, tag="rw",
                                name="rw")
                nc.vector.reciprocal_approx_fast(rw[:], dw[:])
                nc.vector.tensor_mul(out[:, P(0)], raw[:, 0:BC],
                                     rw[:, 0:BC])
                for g in (1, 2):
                    bb = rw[:, g * BC:(g + 1) * BC].rearrange(
                        "p (u b) -> p u b", u=1).broadcast_to([128, 3, BC])
                    nc.vector.tensor_mul(
                        out[:, GSL[g]].rearrange("p (i b) -> p i b", i=3),
                        raw[:, GSL[g]].rearrange("p (i b) -> p i b", i=3), bb)
                nc.vector.tensor_mul(out[:, P(7)], raw[:, 7 * BC:],
                                     rw[:, 3 * BC:])

            def mv_silu(psg, out, at, bt, bias):
                """out = sigmoid(a*inv+b)[blade] * (ps + bias on blade0)."""
                sqw = npool.tile([128, 6 * BC], dt.float16, tag="sqw",
                                 name="sqw")
                sq7 = npool.tile([128, BC], dt.float16, tag="sq7",
                                 name="sq7")
                nc.scalar.activation(sqw[:, 0:3 * BC], psg[1][:], AF.Square)
                nc.scalar.activation(sqw[:, 3 * BC:], psg[2][:], AF.Square)
                nc.scalar.activation(sq7[:], psg[3][:], AF.Square)
                sraw = npool.tile([128, NB * BC], dt.float16, tag="sraw",
                                  name="sraw")
                raw0 = sraw[:, 0:BC]
                nc.scalar.activation(sraw[:, 0:BC], psg[0][:], AF.Identity,
                                     bias=bias)
                nc.scalar.copy(sraw[:, BC:4 * BC], psg[1][:])
                nc.scalar.copy(sraw[:, 4 * BC:7 * BC], psg[2][:])
                nc.scalar.copy(sraw[:, 7 * BC:], psg[3][:])
                q12 = npool.tile([128, 2 * BC], dt.float16, tag="q12",
                                 name="q12")
                sqp = sqw[:].rearrange("p (pl b) -> p pl b", pl=6)
                qp = q12[:].rearrange("p (pl b) -> p pl b", pl=2)
                nc.vector.tensor_add(qp, sqp[:, 0:4:3, :], sqp[:, 1:5:3, :])
                nc.vector.tensor_add(qp, qp, sqp[:, 2:6:3, :])
                invs = [raw0, q12[:, 0:BC], q12[:, BC:], sq7[:]]
                gw = npool.tile([128, 4 * BC], dt.float16, tag="gw",
                                name="gw")
                for g in range(4):
                    nc.scalar.activation(gw[:, g * BC:(g + 1) * BC],
                                         invs[g], AF.Sigmoid,
                                         bias=bt[:, g:g + 1],
                                         scale=at[:, g:g + 1])
                nc.vector.tensor_mul(out[:, P(0)], raw0, gw[:, 0:BC])
                for g in (1, 2):
                    bb = gw[:, g * BC:(g + 1) * BC].rearrange(
                        "p (u b) -> p u b", u=1).broadcast_to([128, 3, BC])
                    nc.vector.tensor_mul(
                        out[:, GSL[g]].rearrange("p (i b) -> p i b", i=3),
                        sraw[:, GSL[g]].rearrange("p (i b) -> p i b", i=3),
                        bb)
                nc.vector.tensor_mul(out[:, P(7)], sraw[:, 7 * BC:],
                                     gw[:, 3 * BC:])

            # ---- phase 1-4 weight DMAs, issued up front in consumption
            # order: xT + lr1w first so phase 1 starts ASAP ----------------
            Ht = {}
            H2t = {}
            with tc.tile_pool(name="xacts", bufs=1) as xpool, \
                 tc.tile_pool(name="w_s1", bufs=1) as w1pool:
                xa = xpool.tile([128, KT_IN * NB * BC], dt.float16,
                                tag="X", name="X")
                nc.sync.dma_start(xa[:], xT)
                Xt = {kt: xa[:, kt * NB * BC:(kt + 1) * NB * BC]
                      for kt in range(KT_IN)}
                lr1t = lin_slicer(
                    wload(w1pool, "lr1", lr1w, KT_IN * 4 * NIN), NIN)
                ll1t = lin_slicer(
                    wload(w1pool, "ll1", ll1w, KT_IN * 4 * HID), HID)
                w1t, w1nt = gp_slicers(
                    wload(w1pool, "w1", w1w, KT_IN * NPATHS * HID, chunks=4),
                    wload(w1pool, "w1nn", w1n, KT_IN * NNEG * HID, chunks=2),
                    HID)
                lrgt = wload(w3apool, "lrg", lrgw, KT_HID * 4 * HID, chunks=2)
                lrgsl = lin_slicer(lrgt, HID)

                # xr = normalization(lr1(x))
                XRt = {}
                for mt in range(MT_IN):
                    ps = alloc_ps(f"lr1_{mt}")
                    em = RegionEmitter({g: KT_IN for g in range(4)})
                    emit_lin(em, ps, lr1t, Xt, KT_IN, mt, NIN)
                    em.done()
                    xr = xpool.tile([128, NB * BC], dt.float16,
                                    tag=f"XR_{mt}", name=f"XR_{mt}")
                    normalize(ps, xr, n1sat[mt], n1cbt[mt])
                    XRt[mt] = xr

                # h = silu((ll1(x) + fcgp(x, xr, w1)) / sqrt2)
                Qs = {kt: build_q(Xt[kt], XRt[kt]) for kt in range(KT_IN)}
                Qpl = {kt: Qs[kt][:].rearrange("p (pl b) -> p pl b", pl=NQPL)
                       for kt in range(KT_IN)}
                for mt in range(MT_HID):
                    ps = alloc_ps(f"h_{mt}")
                    totals = {g: KT_IN * (1 + len(GP_SETS_BY_GRADE[g]))
                              for g in range(4)}
                    em = RegionEmitter(totals)
                    emit_lin(em, ps, ll1t, Xt, KT_IN, mt, HID)
                    for kt in range(KT_IN):
                        emit_gp_kt(em, ps, w1t, w1nt, Qpl[kt], kt, mt, HID)
                    em.done()
                    h = hpool.tile([128, NB * BC], dt.float16,
                                   tag=f"H_{mt}", name=f"H_{mt}")
                    mv_silu(ps, h, actat[mt], actbt[mt], b1t[mt])
                    Ht[mt] = h

            # ---- phases 3-6 (phase 4-6 weights stream in freed space) -----
            w2pool = top.enter_context(tc.tile_pool(name="w_s2", bufs=1))
            llgsl = lin_slicer(
                wload(w2pool, "llg", llgw, KT_HID * 4 * HID, chunks=2), HID)
            dwt = wload(w2pool, "dwg", dwg, MT_HID * NPATHS * 128, chunks=4)
            dnt = wload(w2pool, "dwgn", dwgn, MT_HID * NNEG * 128, chunks=2)
            lr2t = lin_slicer(
                wload(w2pool, "lr2", lr2w, KT_HID * 4 * HID, chunks=2), HID)
            w2t, w2nt = gp_slicers(
                wload(w2pool, "w2", w2w, KT_HID * NPATHS * NOUT, chunks=2),
                wload(w2pool, "w2nn", w2n, KT_HID * NNEG * NOUT), NOUT)
            ll2t = lin_slicer(
                wload(w2pool, "ll2", ll2w, KT_HID * 4 * NOUT), NOUT)

            def dwgsl(ct, t):
                base = (ct * NPATHS + t) * 128
                return dwt[:, base:base + 128]

            def dwgsln(ct, t):
                base = (ct * NNEG + NEG_SLOT[t]) * 128
                return dnt[:, base:base + 128]

            with tc.tile_pool(name="hracts", bufs=1) as hrpool:
                # hr = normalization(lrg(h))
                HRt = {}
                for mt in range(MT_HID):
                    ps = alloc_ps(f"lrg_{mt}")
                    em = RegionEmitter({g: KT_HID for g in range(4)})
                    emit_lin(em, ps, lrgsl, Ht, KT_HID, mt, HID)
                    em.done()
                    hr = hrpool.tile([128, NB * BC], dt.float16,
                                     tag=f"HR_{mt}", name=f"HR_{mt}")
                    normalize(ps, hr, ngsat[mt], ngcbt[mt])
                    HRt[mt] = hr

                # h2 = silu((llg(h) + cw_gp(h, hr, wg)) / sqrt2)
                for mt in range(MT_HID):
                    ps2 = alloc_ps(f"h2_{mt}")
                    totals = {g: KT_HID + len(GP_SETS_BY_GRADE[g])
                              for g in range(4)}
                    em = RegionEmitter(totals)
                    emit_lin(em, ps2, llgsl, Ht, KT_HID, mt, HID)
                    q = build_q(Ht[mt], HRt[mt])
                    qpl = q[:].rearrange("p (pl b) -> p pl b", pl=NQPL)
                    for g in range(4):
                        gbase = GRADE_SLICES[g][0]
                        for (t, (j0, L, o0, st, sgn)) in GP_SETS_BY_GRADE[g]:
                            lhs = (dwgsl if sgn > 0 else dwgsln)(mt, t)
                            r0 = j0 - gbase
                            em.mm(g, ps2[g][:, r0 * BC:(r0 + L) * BC],
                                  lhs, plane_sel(qpl, o0, L, st))
                    em.done()
                    h2 = hpool.tile([128, NB * BC], dt.float16,
                                    tag=f"H2_{mt}", name=f"H2_{mt}")
                    mv_silu(ps2, h2, actat[mt], actbt[mt], bgt[mt])
                    H2t[mt] = h2

            # hr2 = normalization(lr2(h2))
            HR2t = {}
            for u in range(MT_HID):
                ps = alloc_ps(f"lr2_{u}")
                em = RegionEmitter({g: KT_HID for g in range(4)})
                emit_lin(em, ps, lr2t, H2t, KT_HID, u, HID)
                em.done()
                hr2 = hpool.tile([128, NB * BC], dt.float16,
                                 tag=f"H_{u}", name=f"HR2_{u}")
                normalize(ps, hr2, n2sat[u], n2cbt[u])
                HR2t[u] = hr2

            # out = (ll2(h2) + fcgp(h2, hr2, w2)) / sqrt2
            ps = alloc_ps("out")
            totals = {g: KT_HID * (1 + len(GP_SETS_BY_GRADE[g]))
                      for g in range(4)}
            em = RegionEmitter(totals)
            emit_lin(em, ps, ll2t, H2t, KT_HID, 0, NOUT)
            for kt in range(KT_HID):
                q = build_q(H2t[kt], HR2t[kt])
                qpl = q[:].rearrange("p (pl b) -> p pl b", pl=NQPL)
                emit_gp_kt(em, ps, w2t, w2nt, qpl, kt, 0, NOUT)
            em.done()
            outs = auxpool.tile([128, NB * BC], dt.float32, tag="outs",
                                name="outs")
            nc.scalar.activation(outs[:, 0:BC], ps[0][:], AF.Identity,
                                 bias=b2t[0])
            nc.scalar.activation(outs[:, BC:4 * BC], ps[1][:], AF.Identity)
            nc.scalar.activation(outs[:, 4 * BC:7 * BC], ps[2][:],
                                 AF.Identity)
            nc.scalar.activation(outs[:, 7 * BC:], ps[3][:], AF.Identity)
            nc.sync.dma_start(outd[0:128],
                              outs[:].rearrange("p (i b) -> p i b", i=NB))

    nc.compile()
    return nc


_PROGRAM = None


def _get_program():
    global _PROGRAM
    if _PROGRAM is None:
        _PROGRAM = build_program()
    return _PROGRAM


def kernel(**inputs):
    from concourse.bass_utils import run_bass_kernel_spmd

    nc = _get_program()
    in_maps = prep_in_maps(inputs)
    res = run_bass_kernel_spmd(nc, in_maps, core_ids=list(range(NCORES)))
    return assemble(res.results)


if __name__ == "__main__":
    nmm = sum(len(TERM_SETS[t]) for t in range(NPATHS))
    print("NEG_TRIPLES:", NEG_TRIPLES)
    print("term-set MMs per (kt,mt):", nmm)


# revision 25
# speedup vs baseline: 1.0272x; 1.0272x over previous
"""Trainium2 Bass kernel for nn_CliffordFourierHead (CGENN-style Clifford net).

Network (per reference): B=1024, IN=256, HID=512, OUT=128, Cl(3,0), 8 blades.
  fcgp1 -> MVSiLU -> channel-wise steerable GP -> MVSiLU -> fcgp2

Strategy (v3):
  - Pure batch data-parallelism over 8 NeuronCores (128 batch rows each).
  - All weights host-transposed to the exact [128, cols] SBUF layout ->
    contiguous multi-KB DMA lines at full HBM bandwidth; negated GP-weight
    copies (for Cayley minus signs) shipped from host too.
  - Weight DMAs for phases 1-4 issued up front; phase 5-6 weights stream
    during phases 3-4 into the space freed by the phase-1-2 pool.
  - One [128, 8*BC] f32 PSUM tile per accumulation group (exactly 2 banks:
    [g0|g1] and [g2|g3] each 512 f32) -> single-op evacuation, 4 groups in
    flight for deep matmul pipelining.
  - Geometric products: mega product tile Q[i,k] = x_i * xr_k on DVE; Cayley
    contraction via TensorE matmuls with strided plane-selection rhs APs.
  - Channel-wise steerable GP as diagonal-matrix matmuls.
  - Normalization/MVSiLU read PSUM directly for squares; affine norm ops on
    ScalarE to offload DVE.

Self-contained: shapes and the Cl(3,0) Cayley table are derived inline.
"""

import contextlib
import math

import numpy as np

NCORES = 8
B, NIN, HID, NOUT = 1024, 256, 512, 128
BC = B // NCORES  # 128 batch rows per core
NB = 8
KT_IN, KT_HID = NIN // 128, HID // 128  # 2, 4
MT_IN, MT_HID, MT_OUT = NIN // 128, HID // 128, NOUT // 128  # 2, 4, 1
GRADE_SLICES = [(0, 1), (1, 4), (4, 7), (7, 8)]
EPS = 1e-6
ISQ2 = 1.0 / math.sqrt(2.0)


def _build_cayley():
    masks = sorted(range(NB), key=lambda m: (bin(m).count("1"), m))
    pos = {m: i for i, m in enumerate(masks)}
    cay = np.zeros((NB, NB, NB), dtype=np.float32)
    for i, mi in enumerate(masks):
        for k, mk in enumerate(masks):
            a, s = mi >> 1, 0
            while a:
                s += bin(a & mk).count("1")
                a >>= 1
            cay[i, pos[mi ^ mk], k] = -1.0 if (s & 1) else 1.0
    triples = []
    for gi in range(4):
        for gj in range(4):
            for gk in range(4):
                (i0, i1), (j0, j1), (k0, k1) = (
                    GRADE_SLICES[gi], GRADE_SLICES[gj], GRADE_SLICES[gk])
                if np.any(cay[i0:i1, j0:j1, k0:k1] != 0):
                    triples.append((gi, gj, gk))
    return cay, triples


CAY, TRIPLES = _build_cayley()
NPATHS = len(TRIPLES)  # 20

# Per triple t: {j: [(i, k, sign), ...]}
TRIPLE_TERMS = []
for t, (gi, gj, gk) in enumerate(TRIPLES):
    (i0, i1), (k0, k1) = GRADE_SLICES[gi], GRADE_SLICES[gk]
    d = {}
    for i in range(i0, i1):
        for k in range(k0, k1):
            j = int(np.nonzero(CAY[i, :, k])[0][0])
            if GRADE_SLICES[gj][0] <= j < GRADE_SLICES[gj][1]:
                d.setdefault(j, []).append((i, k, float(CAY[i, j, k])))
    TRIPLE_TERMS.append(d)


def _build_term_sets():
    """Per triple: list of matmul term-sets (j0, L, plane0, plane_step, sign)."""
    all_sets = []
    for t in range(NPATHS):
        terms = []
        for j, lst in TRIPLE_TERMS[t].items():
            for (i, k, s) in lst:
                terms.append((j, i * 8 + k, s))
        sets = []
        for sgn in (1.0, -1.0):
            pool = sorted(x for x in terms if x[2] == sgn)
            while pool:
                j0, o0, _ = pool.pop(0)
                run = [(j0, o0)]
                step = None
                while True:
                    pick = None
                    for c in pool:
                        if c[0] != run[-1][0] + 1:
                            continue
                        st = c[1] - run[-1][1]
                        if step is None or st == step:
                            pick, pstep = c, st
                            break
                    if pick is None:
                        break
                    step = pstep
                    pool.remove(pick)
                    run.append((pick[0], pick[1]))
                sets.append((run[0][0], len(run), run[0][1], step or 0, sgn))
        all_sets.append(sets)
    return all_sets


TERM_SETS = _build_term_sets()
# Pre-reduced planes: t4 (1,0,1) planes 9,18,27 -> plane 64 (+);
# t10 (2,0,2) planes 36,45,54 -> plane 65 (negated weight handles sign).
TERM_SETS[4] = [(0, 1, 64, 0, 1.0)]
TERM_SETS[10] = [(0, 1, 65, 0, -1.0)]
NQPL = 66
NEG_TRIPLES = sorted({t for t in range(NPATHS)
                      if any(s[4] < 0 for s in TERM_SETS[t])})
NEG_SLOT = {t: n for n, t in enumerate(NEG_TRIPLES)}
NNEG = len(NEG_TRIPLES)


# ----------------------------------------------------------------------------
# Host-side prep: everything in final [128, cols] SBUF layout
# ----------------------------------------------------------------------------
def prep_in_maps(inputs):
    f16, f32 = np.float16, np.float32

    def lin_w(w, scale=1.0):
        # [m, n, 4] -> [128, nkt*4*m]
        m, n, _ = np.asarray(w).shape
        wt = np.transpose(np.asarray(w, f32), (1, 2, 0))  # [n, 4, m]
        wt = wt.reshape(n // 128, 128, 4, m).transpose(1, 0, 2, 3)
        return np.ascontiguousarray((wt * scale).reshape(128, -1)).astype(f16)

    def gp_w(w, scale):
        # [m, n, 20] -> pos [128, nkt*20*m], neg [128, nkt*12*m]
        m, n, _ = np.asarray(w).shape
        wt = np.transpose(np.asarray(w, f32), (1, 2, 0)) * scale  # [n, 20, m]
        wt = wt.reshape(n // 128, 128, NPATHS, m).transpose(1, 0, 2, 3)
        pos = np.ascontiguousarray(wt.reshape(128, -1)).astype(f16)
        neg = np.ascontiguousarray(
            (-wt[:, :, NEG_TRIPLES, :]).reshape(128, -1)).astype(f16)
        return pos, neg

    def sig(a):
        return 1.0 / (1.0 + np.exp(-np.asarray(a, f32)))

    x = np.asarray(inputs["x"], f32)

    c = {}
    c["lr1w"] = lin_w(inputs["lr1_w"])
    c["ll1w"] = lin_w(inputs["ll1_w"], ISQ2)
    c["lrgw"] = lin_w(inputs["lrg_w"])
    c["llgw"] = lin_w(inputs["llg_w"], ISQ2)
    c["lr2w"] = lin_w(inputs["lr2_w"])
    c["ll2w"] = lin_w(inputs["ll2_w"], ISQ2)
    c["w1w"], c["w1n"] = gp_w(inputs["w1"], ISQ2)
    c["w2w"], c["w2n"] = gp_w(inputs["w2"], ISQ2)

    # channel-wise GP weights as diagonal matrices, [128, ct*20*128] (+neg)
    wg = np.asarray(inputs["wg"], f32) * ISQ2  # [HID, 20]
    wgv = wg.reshape(MT_HID, 128, NPATHS)
    dwg = np.zeros((128, MT_HID, NPATHS, 128), f32)
    idx = np.arange(128)
    for ct in range(MT_HID):
        for t in range(NPATHS):
            dwg[idx, ct, t, idx] = wgv[ct, :, t]
    c["dwg"] = np.ascontiguousarray(dwg.reshape(128, -1)).astype(f16)
    dwgn = -dwg[:, :, NEG_TRIPLES, :]
    c["dwgn"] = np.ascontiguousarray(dwgn.reshape(128, -1)).astype(f16)

    cols = []   # [128, w] blocks, order must match PARAM_LAYOUT

    def addp(arr):
        cols.append(np.asarray(arr, f32).reshape(128, -1))

    for nm, a, kt in (("n1", inputs["n1_a"], KT_IN),
                      ("ng", inputs["ng_a"], KT_HID),
                      ("n2", inputs["n2_a"], KT_HID)):
        sa = sig(a).reshape(kt, 128, 4)
        cb = (1.0 + EPS) - sa
        for u in range(kt):
            addp(sa[u])
            addp(cb[u])
    aa = np.asarray(inputs["act_a"], f32).reshape(MT_HID, 128, 4)
    ab = np.asarray(inputs["act_b"], f32).reshape(MT_HID, 128, 4)
    for u in range(MT_HID):
        addp(aa[u])
        addp(ab[u])
    addp((np.asarray(inputs["ll1_b"], f32) * ISQ2).reshape(MT_HID, 128).T)
    addp((np.asarray(inputs["llg_b"], f32) * ISQ2).reshape(MT_HID, 128).T)
    addp((np.asarray(inputs["ll2_b"], f32) * ISQ2).reshape(MT_OUT, 128).T)
    c["prm"] = np.ascontiguousarray(np.concatenate(cols, axis=1))

    in_maps = []
    for cid in range(NCORES):
        xc = x[cid * BC:(cid + 1) * BC]  # [BC, 256, 8]
        xt = np.transpose(xc, (1, 2, 0)).reshape(KT_IN, 128, NB, BC)
        xt = xt.transpose(1, 0, 2, 3).reshape(128, -1)  # [128, kt*8*BC]
        m = dict(c)
        m["xT"] = np.ascontiguousarray(xt).astype(f16)
        in_maps.append(m)
    return in_maps


def assemble(results):
    out = np.empty((B, NOUT, NB), np.float32)
    for cid in range(NCORES):
        od = np.asarray(results[cid]["outd"])  # [128, 8, BC]
        out[cid * BC:(cid + 1) * BC] = od.transpose(2, 0, 1)
    return out


# ----------------------------------------------------------------------------
# Device program (identical on all 8 cores)
# ----------------------------------------------------------------------------
def build_program():
    import concourse.mybir as mybir
    import concourse.tile as tile
    from concourse import bacc

    dt = mybir.dt
    AF = mybir.ActivationFunctionType
    OP = mybir.AluOpType

    nc = bacc.Bacc("TRN2", target_bir_lowering=False, debug=False,
                   num_devices=NCORES)

    def din(name, cols, dtype=dt.float16):
        return nc.dram_tensor(name, [128, cols], dtype,
                              kind="ExternalInput").ap()

    xT = din("xT", KT_IN * NB * BC)
    lr1w = din("lr1w", KT_IN * 4 * NIN)
    ll1w = din("ll1w", KT_IN * 4 * HID)
    w1w = din("w1w", KT_IN * NPATHS * HID)
    w1n = din("w1n", KT_IN * NNEG * HID)
    lrgw = din("lrgw", KT_HID * 4 * HID)
    llgw = din("llgw", KT_HID * 4 * HID)
    dwg = din("dwg", MT_HID * NPATHS * 128)
    dwgn = din("dwgn", MT_HID * NNEG * 128)
    lr2w = din("lr2w", KT_HID * 4 * HID)
    w2w = din("w2w", KT_HID * NPATHS * NOUT)
    w2n = din("w2n", KT_HID * NNEG * NOUT)
    ll2w = din("ll2w", KT_HID * 4 * NOUT)
    prm = din("prm", 121, dt.float32)
    outd = nc.dram_tensor("outd", [128, NB, BC], dt.float32,
                          kind="ExternalOutput").ap()

    P = lambda j: slice(j * BC, (j + 1) * BC)
    GSL = [slice(j0 * BC, j1 * BC) for (j0, j1) in GRADE_SLICES]

    with tile.TileContext(nc) as tc:
        top = contextlib.ExitStack()
        with top:
            ppool = top.enter_context(tc.tile_pool(name="params", bufs=1))
            w3apool = top.enter_context(tc.tile_pool(name="w_s3a", bufs=1))
            npool = top.enter_context(tc.tile_pool(name="nsc", bufs=2))
            qpool = top.enter_context(tc.tile_pool(name="q", bufs=2))
            pspool = top.enter_context(
                tc.tile_pool(name="psum", bufs=8, space="PSUM"))
            hpool = top.enter_context(tc.tile_pool(name="hacts", bufs=1))
            auxpool = top.enter_context(tc.tile_pool(name="aux", bufs=1))

            def wload(pool, name, src, cols, chunks=1):
                t = pool.tile([128, cols], dt.float16, tag=name, name=name)
                step = cols // chunks
                for ci in range(chunks):
                    nc.sync.dma_start(t[:, ci * step:(ci + 1) * step],
                                      src[:, ci * step:(ci + 1) * step])
                return t

            def lin_slicer(t, mtot):
                def sl(kt, g, mt):
                    base = (kt * 4 + g) * mtot + mt * 128
                    return t[:, base:base + 128]
                return sl

            def gp_slicers(tp, tn, mtot):
                def sl(kt, tt, mt):
                    base = (kt * NPATHS + tt) * mtot + mt * 128
                    return tp[:, base:base + 128]

                def sln(kt, tt, mt):
                    base = (kt * NNEG + NEG_SLOT[tt]) * mtot + mt * 128
                    return tn[:, base:base + 128]
                return sl, sln

            prmt = ppool.tile([128, 121], dt.float32, tag="prm", name="prm")
            nc.sync.dma_start(prmt[:], prm)
            PN1, PNG, PN2, PACT, PB1, PBG, PB2 = 0, 16, 48, 80, 112, 116, 120

            n1sat = {u: prmt[:, PN1 + 8 * u:PN1 + 8 * u + 4]
                     for u in range(KT_IN)}
            n1cbt = {u: prmt[:, PN1 + 8 * u + 4:PN1 + 8 * u + 8]
                     for u in range(KT_IN)}
            ngsat = {u: prmt[:, PNG + 8 * u:PNG + 8 * u + 4]
                     for u in range(KT_HID)}
            ngcbt = {u: prmt[:, PNG + 8 * u + 4:PNG + 8 * u + 8]
                     for u in range(KT_HID)}
            n2sat = {u: prmt[:, PN2 + 8 * u:PN2 + 8 * u + 4]
                     for u in range(KT_HID)}
            n2cbt = {u: prmt[:, PN2 + 8 * u + 4:PN2 + 8 * u + 8]
                     for u in range(KT_HID)}
            actat = {u: prmt[:, PACT + 8 * u:PACT + 8 * u + 4]
                     for u in range(MT_HID)}
            actbt = {u: prmt[:, PACT + 8 * u + 4:PACT + 8 * u + 8]
                     for u in range(MT_HID)}
            b1t = {u: prmt[:, PB1 + u:PB1 + u + 1] for u in range(MT_HID)}
            bgt = {u: prmt[:, PBG + u:PBG + u + 1] for u in range(MT_HID)}
            b2t = {0: prmt[:, PB2:PB2 + 1]}

            GP_SETS_BY_GRADE = {g: [(t, s) for t in range(NPATHS)
                                    if TRIPLES[t][1] == g
                                    for s in TERM_SETS[t]]
                                for g in range(4)}

            class RegionEmitter:
                """start on first / stop on last matmul per psum region."""

                def __init__(self, totals):
                    self.totals = dict(totals)
                    self.seen = {}

                def mm(self, reg, dst, lhs, rhs):
                    i = self.seen.get(reg, 0)
                    nc.tensor.matmul(dst, lhs, rhs, start=(i == 0),
                                     stop=(i == self.totals[reg] - 1))
                    self.seen[reg] = i + 1

                def done(self):
                    assert self.seen == self.totals, (self.seen, self.totals)

            GW = [1, 3, 3, 1]  # blade planes per grade

            def alloc_ps(nm):
                """Four per-grade psum tiles (one bank each) — separate banks
                so interleaved start=True bank-clears stay independent."""
                return [pspool.tile([128, GW[g] * BC], dt.float32, tag="psg",
                                    name=f"ps_{nm}_{g}") for g in range(4)]

            def plane_sel(qpl, o0, L, st):
                if L == 1:
                    return qpl[:, o0:o0 + 1, :]
                last = o0 + st * (L - 1)
                stop = last + 1 if st > 0 else (last - 1 if last >= 1 else None)
                return qpl[:, o0:stop:st, :]

            def build_q(xt, xrt):
                """Mega product tile Q[i*8+k] = x_i * xr_k, [128, 8192]."""
                q = qpool.tile([128, NQPL * BC], dt.float16, tag="Q",
                               name="Q")
                for half in range(2):
                    i0 = half * 4
                    a = xt[:, i0 * BC:(i0 + 4) * BC].rearrange(
                        "p (i u b) -> p i u b", i=4, u=1).broadcast_to(
                        [128, 4, 8, BC])
                    bb = xrt[:].rearrange(
                        "p (u k b) -> p u k b", u=1, k=8).broadcast_to(
                        [128, 4, 8, BC])
                    dst = q[:, i0 * 8 * BC:(i0 + 4) * 8 * BC].rearrange(
                        "p (i k b) -> p i k b", i=4, k=8)
                    nc.vector.tensor_mul(dst, a, bb)
                # pre-reduce t4 (9+18+27 -> 64) and t10 (36+45+54 -> 65)
                qv = q[:].rearrange("p (pl b) -> p pl b", pl=NQPL)
                nc.vector.tensor_add(qv[:, 64:66, :], qv[:, 9:37:27, :],
                                     qv[:, 18:46:27, :])
                nc.vector.tensor_add(qv[:, 64:66, :], qv[:, 64:66, :],
                                     qv[:, 27:55:27, :])
                return q

            def emit_lin(em, psg, wsl, xts, nkt, mt, mtot):
                for kt in range(nkt):
                    for g in range(4):
                        em.mm(g, psg[g][:], wsl(kt, g, mt),
                              xts[kt][:, GSL[g]])

            def emit_gp_kt(em, psg, wsl, wsln, qpl, kt, mt, mtot):
                for g in range(4):
                    gbase = GRADE_SLICES[g][0]
                    for (t, (j0, L, o0, st, sgn)) in GP_SETS_BY_GRADE[g]:
                        lhs = (wsl if sgn > 0 else wsln)(kt, t, mt)
                        r0 = j0 - gbase
                        em.mm(g, psg[g][:, r0 * BC:(r0 + L) * BC],
                              lhs, plane_sel(qpl, o0, L, st))

            def normalize(psg, out, sat, cbt):
                """out = ps / (sa*sqrt(q)+cb). Evacuate psum to SBUF f16
                immediately (frees the banks fast for the next group), then
                run the whole chain from SBUF."""
                raw = npool.tile([128, NB * BC], dt.float16, tag="nraw",
                                 name="nraw")
                sqw = npool.tile([128, 6 * BC], dt.float16, tag="sqw",
                                 name="sqw")
                qw = npool.tile([128, 4 * BC], dt.float16, tag="qw",
                                name="qw")
                nc.scalar.activation(qw[:, 0:BC], psg[0][:], AF.Square)
                nc.scalar.activation(sqw[:, 0:3 * BC], psg[1][:], AF.Square)
                nc.scalar.activation(sqw[:, 3 * BC:], psg[2][:], AF.Square)
                nc.scalar.activation(qw[:, 3 * BC:], psg[3][:], AF.Square)
                nc.scalar.copy(raw[:, 0:BC], psg[0][:])
                nc.scalar.copy(raw[:, BC:4 * BC], psg[1][:])
                nc.vector.tensor_copy(raw[:, 4 * BC:7 * BC], psg[2][:])
                nc.vector.tensor_copy(raw[:, 7 * BC:], psg[3][:])
                sqp = sqw[:].rearrange("p (pl b) -> p pl b", pl=6)
                qp = qw[:, BC:3 * BC].rearrange("p (pl b) -> p pl b", pl=2)
                nc.vector.tensor_add(qp, sqp[:, 0:4:3, :], sqp[:, 1:5:3, :])
                nc.vector.tensor_add(qp, qp, sqp[:, 2:6:3, :])
                nrmw = npool.tile([128, 4 * BC], dt.float16, tag="nrmw",
                                  name="nrmw")
                nc.scalar.activation(nrmw[:], qw[:], AF.Sqrt)
                dw = npool.tile([128, 4 * BC], dt.float32, tag="dw",
                                name="dw")
                for g in range(4):
                    eng = nc.scalar if g % 2 == 0 else None
                    if eng is not None:
                        eng.activation(dw[:, g * BC:(g + 1) * BC],
                                       nrmw[:, g * BC:(g + 1) * BC],
                                       AF.Identity, bias=cbt[:, g:g + 1],
                                       scale=sat[:, g:g + 1])
                    else:
                        nc.vector.tensor_scalar(
                            dw[:, g * BC:(g + 1) * BC],
                            nrmw[:, g * BC:(g + 1) * BC],
                            sat[:, g:g + 1], cbt[:, g:g + 1],
                            OP.mult, OP.add)
                rw = npool.tile([128, 4 * BC], dt.float32, tag="rw",
                                name="rw")
                nc.vector.reciprocal_approx_fast(rw[:], dw[:])
                nc.vector.tensor_mul(out[:, P(0)], raw[:, 0:BC],
                                     rw[:, 0:BC])
                for g in (1, 2):
                    bb = rw[:, g * BC:(g + 1) * BC].rearrange(
                        "p (u b) -> p u b", u=1).broadcast_to([128, 3, BC])
                    nc.vector.tensor_mul(
                        out[:, GSL[g]].rearrange("p (i b) -> p i b", i=3),
                        raw[:, GSL[g]].rearrange("p (i b) -> p i b", i=3), bb)
                nc.vector.tensor_mul(out[:, P(7)], raw[:, 7 * BC:],
                                     rw[:, 3 * BC:])

            def mv_silu(psg, out, at, bt, bias):
                """out = sigmoid(a*inv+b)[blade] * (ps + bias on blade0)."""
                sqw = npool.tile([128, 6 * BC], dt.float16, tag="sqw",
                                 name="sqw")
                sq7 = npool.tile([128, BC], dt.float16, tag="sq7",
                                 name="sq7")
                nc.scalar.activation(sqw[:, 0:3 * BC], psg[1][:], AF.Square)
                nc.scalar.activation(sqw[:, 3 * BC:], psg[2][:], AF.Square)
                nc.scalar.activation(sq7[:], psg[3][:], AF.Square)
                sraw = npool.tile([128, NB * BC], dt.float16, tag="sraw",
                                  name="sraw")
                raw0 = sraw[:, 0:BC]
                nc.scalar.activation(sraw[:, 0:BC], psg[0][:], AF.Identity,
                                     bias=bias)
                nc.scalar.copy(sraw[:, BC:4 * BC], psg[1][:])
                nc.scalar.copy(sraw[:, 4 * BC:7 * BC], psg[2][:])
                nc.scalar.copy(sraw[:, 7 * BC:], psg[3][:])
                q12 = npool.tile([128, 2 * BC], dt.float16, tag="q12",
                                 name="q12")
                sqp = sqw[:].rearrange("p (pl b) -> p pl b", pl=6)
                qp = q12[:].rearrange("p (pl b) -> p pl b", pl=2)
                nc.vector.tensor_add(qp, sqp[:, 0:4:3, :], sqp[:, 1:5:3, :])
                nc.vector.tensor_add(qp, qp, sqp[:, 2:6:3, :])
                invs = [raw0, q12[:, 0:BC], q12[:, BC:], sq7[:]]
                gw = npool.tile([128, 4 * BC], dt.float16, tag="gw",
                                name="gw")
                for g in range(4):
                    nc.scalar.activation(gw[:, g * BC:(g + 1) * BC],
                                         invs[g], AF.Sigmoid,
                                         bias=bt[:, g:g + 1],
                                         scale=at[:, g:g + 1])
                nc.vector.tensor_mul(out[:, P(0)], raw0, gw[:, 0:BC])
                for g in (1, 2):
                    bb = gw[:, g * BC:(g + 1) * BC].rearrange(
                        "p (u b) -> p u b", u=1).broadcast_to([128, 3, BC])
                    nc.vector.tensor_mul(
                        out[:, GSL[g]].rearrange("p (i b) -> p i b", i=3),
                        sraw[:, GSL[g]].rearrange("p (i b) -> p i b", i=3),
                        bb)
                nc.vector.tensor_mul(out[:, P(7)], sraw[:, 7 * BC:],
                                     gw[:, 3 * BC:])

            # ---- phase 1-4 weight DMAs, issued up front in consumption
            # order: xT + lr1w first so phase 1 starts ASAP ----------------
            Ht = {}
            H2t = {}
            with tc.tile_pool(name="xacts", bufs=1) as xpool, \
                 tc.tile_pool(name="w_s1", bufs=1) as w1pool:
                xa = xpool.tile([128, KT_IN * NB * BC], dt.float16,
                                tag="X", name="X")
                nc.sync.dma_start(xa[:], xT)
                Xt = {kt: xa[:, kt * NB * BC:(kt + 1) * NB * BC]
                      for kt in range(KT_IN)}
                lr1t = lin_slicer(
                    wload(w1pool, "lr1", lr1w, KT_IN * 4 * NIN), NIN)
                ll1t = lin_slicer(
                    wload(w1pool, "ll1", ll1w, KT_IN * 4 * HID), HID)
                w1t, w1nt = gp_slicers(
                    wload(w1pool, "w1", w1w, KT_IN * NPATHS * HID, chunks=4),
                    wload(w1pool, "w1nn", w1n, KT_IN * NNEG * HID, chunks=2),
                    HID)
                lrgt = wload(w3apool, "lrg", lrgw, KT_HID * 4 * HID, chunks=2)
                lrgsl = lin_slicer(lrgt, HID)

                # xr = normalization(lr1(x))
                XRt = {}
                for mt in range(MT_IN):
                    ps = alloc_ps(f"lr1_{mt}")
                    em = RegionEmitter({g: KT_IN for g in range(4)})
                    emit_lin(em, ps, lr1t, Xt, KT_IN, mt, NIN)
                    em.done()
                    xr = xpool.tile([128, NB * BC], dt.float16,
                                    tag=f"XR_{mt}", name=f"XR_{mt}")
                    normalize(ps, xr, n1sat[mt], n1cbt[mt])
                    XRt[mt] = xr

                # h = silu((ll1(x) + fcgp(x, xr, w1)) / sqrt2)
                Qs = {kt: build_q(Xt[kt], XRt[kt]) for kt in range(KT_IN)}
                Qpl = {kt: Qs[kt][:].rearrange("p (pl b) -> p pl b", pl=NQPL)
                       for kt in range(KT_IN)}
                for mt in range(MT_HID):
                    ps = alloc_ps(f"h_{mt}")
                    totals = {g: KT_IN * (1 + len(GP_SETS_BY_GRADE[g]))
                              for g in range(4)}
                    em = RegionEmitter(totals)
                    emit_lin(em, ps, ll1t, Xt, KT_IN, mt, HID)
                    for kt in range(KT_IN):
                        emit_gp_kt(em, ps, w1t, w1nt, Qpl[kt], kt, mt, HID)
                    em.done()
                    h = hpool.tile([128, NB * BC], dt.float16,
                                   tag=f"H_{mt}", name=f"H_{mt}")
                    mv_silu(ps, h, actat[mt], actbt[mt], b1t[mt])
                    Ht[mt] = h

            # ---- phases 3-6 (phase 4-6 weights stream in freed space) -----
            w2pool = top.enter_context(tc.tile_pool(name="w_s2", bufs=1))
            llgsl = lin_slicer(
                wload(w2pool, "llg", llgw, KT_HID * 4 * HID, chunks=2), HID)
            dwt = wload(w2pool, "dwg", dwg, MT_HID * NPATHS * 128, chunks=4)
            dnt = wload(w2pool, "dwgn", dwgn, MT_HID * NNEG * 128, chunks=2)
            lr2t = lin_slicer(
                wload(w2pool, "lr2", lr2w, KT_HID * 4 * HID, chunks=2), HID)
            w2t, w2nt = gp_slicers(
                wload(w2pool, "w2", w2w, KT_HID * NPATHS * NOUT, chunks=2),
                wload(w2pool, "w2nn", w2n, KT_HID * NNEG * NOUT), NOUT)
            ll2t = lin_slicer(
                wload(w2pool, "ll2", ll2w, KT_HID * 4 * NOUT), NOUT)

            def dwgsl(ct, t):
                base = (ct * NPATHS + t) * 128
                return dwt[:, base:base + 128]

            def dwgsln(ct, t):
                base = (ct * NNEG + NEG_SLOT[t]) * 128
                return dnt[:, base:base + 128]

            with tc.tile_pool(name="hracts", bufs=1) as hrpool:
                # hr = normalization(lrg(h))
                HRt = {}
                for mt in range(MT_HID):
                    ps = alloc_ps(f"lrg_{mt}")
                    em = RegionEmitter({g: KT_HID for g in range(4)})
                    emit_lin(em, ps, lrgsl, Ht, KT_HID, mt, HID)
                    em.done()
                    hr = hrpool.tile([128, NB * BC], dt.float16,
                                     tag=f"HR_{mt}", name=f"HR_{mt}")
                    normalize(ps, hr, ngsat[mt], ngcbt[mt])
                    HRt[mt] = hr

                # h2 = silu((llg(h) + cw_gp(h, hr, wg)) / sqrt2)
                for mt in range(MT_HID):
                    ps2 = alloc_ps(f"h2_{mt}")
                    totals = {g: KT_HID + len(GP_SETS_BY_GRADE[g])
                              for g in range(4)}
                    em = RegionEmitter(totals)
                    emit_lin(em, ps2, llgsl, Ht, KT_HID, mt, HID)
                    q = build_q(Ht[mt], HRt[mt])
                    qpl = q[:].rearrange("p (pl b) -> p pl b", pl=NQPL)
                    for g in range(4):
                        gbase = GRADE_SLICES[g][0]
                        for (t, (j0, L, o0, st, sgn)) in GP_SETS_BY_GRADE[g]:
                            lhs = (dwgsl if sgn > 0 else dwgsln)(mt, t)
                            r0 = j0 - gbase
                            em.mm(g, ps2[g][:, r0 * BC:(r0 + L) * BC],
                                  lhs, plane_sel(qpl, o0, L, st))
                    em.done()
                    h2 = hpool.tile([128, NB * BC], dt.float16,
                                    tag=f"H2_{mt}", name=f"H2_{mt}")
                    mv_silu(ps2, h2, actat[mt], actbt[mt], bgt[mt])
                    H2t[mt] = h2

            # hr2 = normalization(lr2(h2))
            HR2t = {}
            for u in range(MT_HID):
                ps = alloc_ps(f"lr2_{u}")
                em = RegionEmitter({g: KT_HID for g in range(4)})
                emit_lin(em, ps, lr2t, H2t, KT_HID, u, HID)
                em.done()
                hr2 = hpool.tile([128, NB * BC], dt.float16,
                                 tag=f"H_{u}", name=f"HR2_{u}")
                normalize(ps, hr2, n2sat[u], n2cbt[u])
                HR2t[u] = hr2

            # out = (ll2(h2) + fcgp(h2, hr2, w2)) / sqrt2
            ps = alloc_ps("out")
            totals = {g: KT_HID * (1 + len(GP_SETS_BY_GRADE[g]))
                      for g in range(4)}
            em = RegionEmitter(totals)
            emit_lin(em, ps, ll2t, H2t, KT_HID, 0, NOUT)
            for kt in range(KT_HID):
                q = build_q(H2t[kt], HR2t[kt])
                qpl = q[:].rearrange("p (pl b) -> p pl b", pl=NQPL)
                emit_gp_kt(em, ps, w2t, w2nt, qpl, kt, 0, NOUT)
            em.done()
            outs = auxpool.tile([128, NB * BC], dt.float32, tag="outs",
                                name="outs")
            nc.scalar.activation(outs[:, 0:BC], ps[0][:], AF.Identity,
                                 bias=b2t[0])
            nc.scalar.activation(outs[:, BC:4 * BC], ps[1][:], AF.Identity)
            nc.scalar.activation(outs[:, 4 * BC:7 * BC], ps[2][:],
                                 AF.Identity)
            nc.scalar.activation(outs[:, 7 * BC:], ps[3][:], AF.Identity)
            for (a, b) in GRADE_SLICES:
                nc.sync.dma_start(
                    outd[0:128, a:b],
                    outs[:, a * BC:b * BC].rearrange("p (i b) -> p i b",
                                                     i=b - a))

    nc.compile()
    return nc


_PROGRAM = None


def _get_program():
    global _PROGRAM
    if _PROGRAM is None:
        _PROGRAM = build_program()
    return _PROGRAM


def kernel(**inputs):
    from concourse.bass_utils import run_bass_kernel_spmd

    nc = _get_program()
    in_maps = prep_in_maps(inputs)
    res = run_bass_kernel_spmd(nc, in_maps, core_ids=list(range(NCORES)))
    return assemble(res.results)


if __name__ == "__main__":
    nmm = sum(len(TERM_SETS[t]) for t in range(NPATHS))
    print("NEG_TRIPLES:", NEG_TRIPLES)
    print("term-set MMs per (kt,mt):", nmm)
